# revision 1
# baseline (speedup 1.0000x reference)
"""Trainium2 Bass kernel for nn_AlignerOT: batched 1-D entropic OT (Sinkhorn).

Math
----
Per (b,s) problem (2048 of them, 128 points each):
  C[i,j] = 300*(x_i - y_j)^2 ;  NC = -C/eps = -3000*(x_i-y_j)^2
  log-domain Sinkhorn (20 iters) == scaling-form Sinkhorn on the shifted kernel
  K = exp(NC - mu_i - nut_j) with mu_i = row max of NC, nut_j = col max of
  (NC - mu_i).  With v0 = exp(nut),
      u <- 1/(K v) ; v <- 1/(K^T u)        (m = n cancels all 1/m factors)
  reproduces the reference's iterates exactly (in exact arithmetic); the final
  plan is (1/m) * u ∘ K ∘ v.  All scaling residuals stay within ~e^{+-55},
  fp32-safe (verified numerically; naive exp(f/eps) would overflow to e^16599).

Mapping
-------
Data-parallel: 256 problems per core on 8 cores; per problem, NC is built by a
K=3 matmul from host-prepped [x^2, x, 1] / [-3000, 6000y, -3000y^2] stacks,
shifts via free-dim reduce_max + Exp-activation bias, matvecs on TensorE with
per-problem weight loads, reciprocals batched [128, PC] on VectorE.  Per-core
plan sums are AllReduce'd, ot is formed on-chip, and each core emits its
[256,128] slice of X @ ot.
"""

import numpy as np

import concourse.bass as bass
import concourse.mybir as mybir
from concourse import tile
from concourse.bass_utils import run_bass_kernel_spmd

F32 = mybir.dt.float32
AF = mybir.ActivationFunctionType

N_CORES = 8
B, S, D = 8, 256, 128
NPROB = (B * S) // N_CORES      # problems per core
PC = 64                         # problems per chunk
NCHUNK = NPROB // PC
NITER = 20
SCALE = 300.0
EPS = 0.1
CINV = SCALE / EPS              # 3000.0

_CACHED = {}


def build_nc(niter=NITER, nprob=NPROB, pc=PC, n_cores=N_CORES, dbg=False,
             bf16=False, no_cc=False, ginter=2, batch4=False, stbufs=3, pb=2, pt=2):
    import concourse.bacc as bacc

    KDT = mybir.dt.bfloat16 if bf16 else F32
    nchunk = nprob // pc
    nc = bacc.Bacc(
        "TRN2",
        target_bir_lowering=False,
        debug=False,
        enable_asserts=False,
        num_devices=n_cores,
    )
    lhs3 = nc.dram_tensor("lhs3", [3, nprob * D], F32, kind="ExternalInput").ap()
    rhs3 = nc.dram_tensor("rhs3", [3, nprob * D], F32, kind="ExternalInput").ap()
    xt = nc.dram_tensor("xt", [D, nprob], F32, kind="ExternalInput").ap()
    delta = nc.dram_tensor("delta", [D, D], F32, kind="ExternalInput").ap()
    ident = nc.dram_tensor("ident", [D, D], F32, kind="ExternalInput").ap()
    out = nc.dram_tensor("out", [nprob, D], F32, kind="ExternalOutput").ap()
    if dbg:
        dbg_kb = nc.dram_tensor("dbg_kb", [D, pc * D], F32, kind="ExternalOutput").ap()
        dbg_nn = nc.dram_tensor("dbg_nn", [D, pc], F32, kind="ExternalOutput").ap()
        dbg_u = nc.dram_tensor("dbg_u", [D, pc], F32, kind="ExternalOutput").ap()
        dbg_v = nc.dram_tensor("dbg_v", [D, pc], F32, kind="ExternalOutput").ap()
        dbg_acc = nc.dram_tensor("dbg_acc", [D, D], F32, kind="ExternalOutput").ap()
        dbg_ot = nc.dram_tensor("dbg_ot", [D, D], F32, kind="ExternalOutput").ap()

    with tile.TileContext(nc) as tc:
        with (
            tc.tile_pool(name="const", bufs=1) as cpool,
            tc.tile_pool(name="kmat", bufs=2 if (bf16 and ginter == 2) else 1) as kpool,
            tc.tile_pool(name="small", bufs=2) as spool,
            tc.tile_pool(name="stage", bufs=stbufs) as stpool,
            tc.tile_pool(name="acc", bufs=1) as apool,
            tc.tile_pool(name="pbig", bufs=pb, space="PSUM") as pbig,
            tc.tile_pool(name="ptr", bufs=pt, space="PSUM") as ptr,
            tc.tile_pool(name="ps", bufs=1, space="PSUM") as ps,
            tc.tile_pool(name="dram", bufs=2, space="DRAM") as dpool,
        ):
            # ---- constants / inputs resident in SBUF ----
            xt_sb = cpool.tile([D, nprob], F32, tag="xt")
            delta_sb = cpool.tile([D, D], F32, tag="delta")
            id_sb = cpool.tile([D, D], F32, tag="ident")
            nc.sync.dma_start(xt_sb[:], xt[:])
            nc.sync.dma_start(delta_sb[:], delta[:])
            nc.sync.dma_start(id_sb[:], ident[:])

            acc_sb = apool.tile([D, D], F32, tag="acc")
            nc.vector.memset(acc_sb[:], 0.0)
            if bf16:
                id16_sb = cpool.tile([D, D], KDT, tag="id16")
                nc.vector.tensor_copy(id16_sb[:], id_sb[:])
            else:
                id16_sb = id_sb

            # process chunks in pairs so one chunk's matvecs hide the other's
            # vector-engine latencies
            for c0 in range(0, nchunk, ginter):
                pair = [c for c in range(c0, c0 + ginter) if c < nchunk]
                kbs, kas, us, vs = {}, {}, {}, {}
                us16, vs16 = {}, {}
                # ---- precompute kernels K (both layouts) for the pair ----
                for c in pair:
                    kb = kpool.tile([D, pc, D], KDT, tag=f"kb{c % ginter}")
                    ka = kpool.tile([D, pc, D], KDT, tag=f"ka{c % ginter}")
                    negnu = spool.tile([D, pc], F32, tag=f"negnu{c % ginter}")
                    kbs[c], kas[c] = kb, ka
                    if batch4:
                      for p0 in range(0, pc, 4):
                        g0 = c * pc + p0
                        lst = stpool.tile([3, 4 * D], F32, tag="lst")
                        rst = stpool.tile([3, 4 * D], F32, tag="rst")
                        nc.sync.dma_start(lst[:], lhs3[:, g0 * D:(g0 + 4) * D])
                        nc.sync.dma_start(rst[:], rhs3[:, g0 * D:(g0 + 4) * D])
                        psA = pbig.tile([D, 4, D], F32, tag="pbig")
                        for q in range(4):
                            nc.tensor.matmul(
                                psA[:, q, :], lst[:, q * D:(q + 1) * D],
                                rst[:, q * D:(q + 1) * D], start=True, stop=True,
                            )
                        mu = stpool.tile([D, 4], F32, tag="mu")
                        nc.vector.reduce_max(mu[:], psA[:], axis=mybir.AxisListType.X)
                        tmpa = stpool.tile([D, 4, D], F32, tag="tmpa")
                        nc.vector.tensor_tensor(
                            tmpa[:], psA[:], mu[:, :, None].broadcast_to([D, 4, D]),
                            op=mybir.AluOpType.subtract,
                        )
                        psB = ptr.tile([D, 4, D], F32, tag="ptr")
                        for q in range(4):
                            nc.tensor.transpose(psB[:, q, :], tmpa[:, q, :], id_sb[:])
                        nc.vector.reduce_max(
                            negnu[:, p0:p0 + 4], psB[:],
                            axis=mybir.AxisListType.X, negate=True,
                        )
                        tmpb = stpool.tile([D, 4, D], F32, tag="tmpb")
                        nc.vector.tensor_tensor(
                            tmpb[:], psB[:],
                            negnu[:, p0:p0 + 4][:, :, None].broadcast_to([D, 4, D]),
                            op=mybir.AluOpType.add,
                        )
                        nc.scalar.activation(
                            kb[:, p0:p0 + 4, :], tmpb[:], AF.Exp, bias=0.0, scale=1.0,
                        )
                        psK = pbig.tile([D, 4, D], KDT, tag="pbig")
                        for q in range(4):
                            nc.tensor.transpose(
                                psK[:, q, :], kb[:, p0 + q, :], id16_sb[:]
                            )
                        nc.scalar.copy(ka[:, p0:p0 + 4, :], psK[:])
                      pc_done = True
                    else:
                      pc_done = False
                    for p in ([] if pc_done else range(pc)):
                        g = c * pc + p  # global problem index on this core
                        lst = stpool.tile([3, D], F32, tag="lst")
                        rst = stpool.tile([3, D], F32, tag="rst")
                        nc.sync.dma_start(lst[:], lhs3[:, g * D:(g + 1) * D])
                        nc.sync.dma_start(rst[:], rhs3[:, g * D:(g + 1) * D])
                        psA = pbig.tile([D, D], F32, tag="pbig")
                        nc.tensor.matmul(
                            psA[:], lst[:], rst[:],
                            start=True, stop=True,
                        )
                        mu = stpool.tile([D, 1], F32, tag="mu")
                        nc.vector.reduce_max(mu[:], psA[:], axis=mybir.AxisListType.X)
                        tmpa = stpool.tile([D, D], F32, tag="tmpa")
                        nc.vector.tensor_scalar_sub(tmpa[:], psA[:], mu[:])
                        psB = ptr.tile([D, D], F32, tag="ptr")
                        nc.tensor.transpose(psB[:], tmpa[:], id_sb[:])
                        nc.vector.reduce_max(
                            negnu[:, p:p + 1], psB[:],
                            axis=mybir.AxisListType.X, negate=True,
                        )
                        nc.scalar.activation(
                            kb[:, p, :], psB[:], AF.Exp,
                            bias=negnu[:, p:p + 1], scale=1.0,
                        )
                        psA2 = pbig.tile([D, D], KDT, tag="pbig")
                        nc.tensor.transpose(psA2[:], kb[:, p, :], id16_sb[:])
                        nc.scalar.copy(ka[:, p, :], psA2[:])
                    # v0 = exp(nut) = exp(-negnu)
                    v = spool.tile([D, pc], F32, tag=f"v{c % ginter}")
                    u = spool.tile([D, pc], F32, tag=f"u{c % ginter}")
                    us[c], vs[c] = u, v
                    if bf16:
                        v16 = spool.tile([D, pc], KDT, tag=f"v16{c % ginter}")
                        u16 = spool.tile([D, pc], KDT, tag=f"u16{c % ginter}")
                        us16[c], vs16[c] = u16, v16
                        nc.scalar.activation(v16[:], negnu[:], AF.Exp, bias=0.0, scale=-1.0)
                    else:
                        us16[c], vs16[c] = u, v
                        nc.scalar.activation(v[:], negnu[:], AF.Exp, bias=0.0, scale=-1.0)
                    if dbg and c == 0:
                        nc.sync.dma_start(dbg_nn[:], negnu[:])

                # ---- 20 sinkhorn iterations, pure matvec + reciprocal ----
                for t in range(niter):
                    for c in pair:
                        sf = ps.tile([D, pc], F32, tag=f"s{c % ginter}" if ginter > 2 else f"sf{c % ginter}")
                        for p in range(pc):
                            nc.tensor.matmul(
                                sf[:, p:p + 1],
                                kbs[c][:, p, :],
                                vs16[c][:, p:p + 1],
                                start=True, stop=True,
                            )
                        nc.vector.reciprocal(us[c][:], sf[:])
                        if bf16:
                            nc.vector.tensor_copy(us16[c][:], us[c][:])
                    for c in pair:
                        sg = ps.tile([D, pc], F32, tag=f"s{c % ginter}" if ginter > 2 else f"sg{c % ginter}")
                        for p in range(pc):
                            nc.tensor.matmul(
                                sg[:, p:p + 1],
                                kas[c][:, p, :],
                                us16[c][:, p:p + 1],
                                start=True, stop=True,
                            )
                        nc.vector.reciprocal(vs[c][:], sg[:])
                        if bf16:
                            nc.vector.tensor_copy(vs16[c][:], vs[c][:])

                if dbg and c0 == 0:
                    nc.sync.dma_start(dbg_kb[:], kbs[0].tensor.ap())
                    nc.sync.dma_start(dbg_u[:], us[0][:])
                    nc.sync.dma_start(dbg_v[:], vs[0][:])

                # ---- accumulate plan sums: acc += u ∘ K ∘ v ----
                for c in pair:
                    if batch4:
                      for p0 in range(0, pc, 4):
                        tb = stpool.tile([D, 4, D], F32, tag="tb")
                        nc.gpsimd.tensor_tensor(
                            tb[:], kbs[c][:, p0:p0 + 4, :],
                            vs[c][:, p0:p0 + 4][:, :, None].broadcast_to([D, 4, D]),
                            op=mybir.AluOpType.mult,
                        )
                        psP = ptr.tile([D, 4, D], F32, tag="ptr")
                        for q in range(4):
                            nc.tensor.transpose(psP[:, q, :], tb[:, q, :], id_sb[:])
                        tp = stpool.tile([D, 4, D], F32, tag="tp")
                        nc.vector.tensor_tensor(
                            tp[:], psP[:],
                            us[c][:, p0:p0 + 4][:, :, None].broadcast_to([D, 4, D]),
                            op=mybir.AluOpType.mult,
                        )
                        tsum = stpool.tile([D, D], F32, tag="tsum")
                        nc.vector.reduce_sum(
                            tsum[:], tp.transpose([0, 2, 1]),
                            axis=mybir.AxisListType.X,
                        )
                        nc.vector.tensor_tensor(
                            acc_sb[:], acc_sb[:], tsum[:], op=mybir.AluOpType.add
                        )
                    else:
                      for p in range(pc):
                        tb = stpool.tile([D, D], F32, tag="tb")
                        nc.scalar.mul(
                            tb[:], kbs[c][:, p, :], vs[c][:, p:p + 1]
                        )
                        psP = ptr.tile([D, D], F32, tag="ptr")
                        nc.tensor.transpose(psP[:], tb[:], id_sb[:])
                        tp = stpool.tile([D, D], F32, tag="tp")
                        nc.scalar.mul(tp[:], psP[:], us[c][:, p:p + 1])
                        nc.vector.tensor_tensor(
                            acc_sb[:], acc_sb[:], tp[:], op=mybir.AluOpType.add
                        )

            # ---- AllReduce plan sums across cores, form ot ----
            ot_sb = apool.tile([D, D], F32, tag="ot")
            if no_cc:
                nc.vector.tensor_copy(ot_sb[:], acc_sb[:])
            else:
                cc_in = dpool.tile([D, D], F32, tag="ccin")
                cc_out = dpool.tile([D, D], F32, tag="ccout")
                nc.sync.dma_start(cc_in[:], acc_sb[:])
                nc.gpsimd.collective_compute(
                    "AllReduce",
                    mybir.AluOpType.add,
                    replica_groups=[list(range(n_cores))],
                    ins=[cc_in.opt()],
                    outs=[cc_out.opt()],
                )
                nc.sync.dma_start(ot_sb[:], cc_out[:])
            # ot = acc_global * (SCALE / n_problems_total) + delta
            nc.vector.tensor_scalar_mul(ot_sb[:], ot_sb[:], SCALE / (n_cores * nprob))
            nc.vector.tensor_tensor(
                ot_sb[:], ot_sb[:], delta_sb[:], op=mybir.AluOpType.add
            )
            if dbg:
                nc.sync.dma_start(dbg_acc[:], acc_sb[:])
                nc.sync.dma_start(dbg_ot[:], ot_sb[:])

            # ---- out = X @ ot  (per-core slice) ----
            for s0 in range(0, nprob, D):
                m = min(D, nprob - s0)
                pso = pbig.tile([D, D], F32, tag="pbig")
                nc.tensor.matmul(
                    pso[:m, :],
                    xt_sb[:, s0:s0 + m],
                    ot_sb[:],
                    start=True, stop=True,
                )
                ostage = stpool.tile([D, D], F32, tag="ostage")
                nc.scalar.copy(ostage[:m, :], pso[:m, :])
                nc.sync.dma_start(out[s0:s0 + m, :], ostage[:m, :])

    nc.finalize()
    return nc


def _prep_inputs(X, Y, delta_ot, n_cores=N_CORES, nprob=NPROB):
    X2 = np.ascontiguousarray(X.reshape(-1, D), dtype=np.float32)
    Y2 = np.ascontiguousarray(Y.reshape(-1, D), dtype=np.float32)
    delta = np.ascontiguousarray(delta_ot, dtype=np.float32)
    ident = np.eye(D, dtype=np.float32)
    in_maps = []
    for k in range(n_cores):
        sl = slice(k * nprob, (k + 1) * nprob)
        Xc, Yc = X2[sl], Y2[sl]
        lhs3 = np.empty((3, nprob * D), np.float32)
        lhs3[0] = (-CINV * Xc * Xc).reshape(-1)
        lhs3[1] = Xc.reshape(-1)
        lhs3[2] = 1.0
        rhs3 = np.empty((3, nprob * D), np.float32)
        rhs3[0] = 1.0
        rhs3[1] = (2.0 * CINV) * Yc.reshape(-1)
        rhs3[2] = (-CINV) * (Yc * Yc).reshape(-1)
        in_maps.append({
            "lhs3": lhs3,
            "rhs3": rhs3,
            "xt": np.ascontiguousarray(Xc.T),
            "delta": delta,
            "ident": ident,
        })
    return in_maps


def run(inputs, trace=False, bf16=False, batch4=False, ginter=2, **kw):
    key = ("nc", bf16, batch4, ginter)
    if key not in _CACHED:
        _CACHED[key] = build_nc(bf16=bf16, batch4=batch4, ginter=ginter)
    nc = _CACHED[key]
    in_maps = _prep_inputs(inputs["X"], inputs["Y"], inputs["delta_ot"])
    res = run_bass_kernel_spmd(
        nc, in_maps, core_ids=list(range(N_CORES)), trace=trace, **kw
    )
    outs = [res.results[k]["out"] for k in range(N_CORES)]
    full = np.concatenate(outs, axis=0).reshape(B, S, D).astype(np.float32)
    return full, res


def kernel(**inputs):
    full, _ = run(inputs, trace=False, bf16=True, batch4=True, ginter=4)
    return full

